# revision 36
# baseline (speedup 1.0000x reference)
"""Trainium2 Bass kernel: batched multi-head attention.

out[b,h] = softmax(Q[b,h] @ K[b,h].T / sqrt(D)) @ V[b,h]
with B=4, H=16, S=2048, D=64, fp32.

Sharding: the 64 (b,h) pairs are split across 8 NeuronCores, 8 pairs per
core; attention is independent per pair, so no cross-core communication.

Device dataflow per pair:
  1. Host pre-lays inputs (bf16 + fp8 to cut DMA traffic and PE time):
       qt  [128, 2048] bf16: (Q/64)^T (d on partitions) duplicated into
                        partitions 64..127 so K=64-contraction matmuls can
                        run via PE row-tiling. The 1/64 pre-scale puts the
                        score stream u = s/64 in [-0.75, 0.75], the domain
                        of the DVE cubic below.
       kt  [128, 1024] bf16: K^T k-tiles interleaved — k-tile 2t at
                        partitions 0..63, k-tile 2t+1 at 64..127.
       vo  [128, 1040] bf16: 16 chunks of [V_ktile | ones] of width 65 —
                        the ones column makes the PV matmul also produce
                        the softmax denominator for free.
       qt8 [32, 2, 2048] fp8e4, kt8 [32, 2, 2, 128] fp8e4: (Q/8)^T and
                        K/8 for k-tiles 0,1, split into two 32-deep d-half
                        "phases" for DoubleRow matmuls. The /8 pre-scales
                        make fp8 products land directly in the same u =
                        s/64 domain as the bf16 stream.
  2. scores^T[k,q] = K^T.T @ Q^T, one [128, 512] f32 slice per matmul.
     K-tiles 0,1 of every group but the first run as fp8e4 DoubleRow
     (cost 0.5 cycles/row vs 1.0 — the only sub-1.0 rate the PE has),
     trimming PE busy ~7us for a measured +0.8e-2 L2 error, well under
     the 2e-2 gate. Groups are (pair, 512-wide q-chunk); each emits 8
     chunks of 2 slices into a 3-deep ring of 2-bank PSUM tiles.
  3. P^T = Lam^8 * exp(8*u) computed on TWO engines in parallel into a
     per-(pair,qc) persistent bf16 buffer ptg [128, 16*512]:
       - ACT chunks: scalar activation exp (scale=8, bias=8*ln(Lam)); the
         EXP table is preloaded by a dummy activation at t~0 so the
         1283ns table load hides under the initial input DMAs.
       - DVE chunks: custom-DVE op  [(u+A)((u+B)u+C)]^8  — a log-minimax
         factored cubic approximation of Lam*e^u on |u|<=0.6875 raised
         to the 8th power by three chained squarings (8 ALU stages,
         per-element rel err ~9.1e-3; the error largely averages out in
         the softmax).
     Chunks alternate strictly ACT,DVE,ACT,... — the 3-deep PSUM ring
     only has slack when consecutive chunks land on different engines,
     and strict alternation beats greedy balancing by ~1.2us end-to-end.
     The Lam^8 factor cancels in the softmax normalization.
  4. PV with pt STATIONARY: out[q128, 65] = ptg_slice.T @ [V|1] — matmul
     cost is output free size (65), so this orientation is ~4x cheaper
     than [65, 512] outputs. All four q-subtiles of a group accumulate
     start=False into disjoint 65-col ranges of ONE shared PSUM bank,
     zeroed up front by a 54ns fp8-DoubleRow matmul from zero constants
     (skip_group_check). That collapses four 65-wide PSUM drains into a
     single 260-wide copy per group, saving ~15us of combined ACT/DVE
     busy time vs per-qsub drains. PV k-tile order is rotated
     [2..15, 0, 1] so the freshest exp slices are consumed last.
  5. One 260-wide drain per group (greedy ACT/DVE) -> SBUF ob
     [128, 4*65] -> one DMA per group to HBM [qsub, 128, 65] rows; the
     host divides cols 0..63 by col 64 — no transpose needed.

Schedule: per chunk slot the PE runs 2 score slices + 8 PV matmuls (half
a q-subtile) — group g's slots carry group g-1's PV in qsub order
q2,q3,q0,q1, making every slot a uniform ~610ns of PE work. All input
and output DMAs ride the SP queue (pairs prefetched two groups ahead);
the first pair's staging is split across the SP and gpsimd queues and
ordered so the first score matmul starts at the ~2.2us DMA-latency
floor, with constants/table-preload queued behind it. Head: group 0's
first three chunks split their exps 2x512 across both engines to fill
the pipeline. Tail: the last group's four accumulators run k-tiles
{2..13, 0, 1, 14} t-major while the closing exps land, the final exp is
split 384/128 with q3's columns first on ACT, and the three output DMAs
leave on three different queues longest-chain-first.

Cost-model occupancy: PE 159.8us busy (94.9%), DVE 152.7 (90.6%), ACT
148.7 (88.2%); exec 168479ns with the only PE idle at the DMA-latency
head (~2.4us), pipeline fill (~1.7us) and the closing drain+DMA chain
(~3.1us).
"""
import sys

sys.path.insert(0, "/opt/trn_rl_repo")

import numpy as np
import ml_dtypes

import concourse.bacc as bacc
import concourse.bass as bass
import concourse.mybir as mybir
import concourse.dve_ops as dve_ops
from concourse.bass_utils import run_bass_kernel_spmd
from concourse.dve_spec import Spec, Src0, C0, C1, C2, lower as dve_lower, sq
from concourse.dve_spec import _has_src1
from concourse.dve_uop import DveOpSpec
from concourse.tile import TileContext

B, H, S, D = 4, 16, 2048, 64
N_CORES = 8
PAIRS = B * H              # 64 independent (b, h) attention problems
PPC = PAIRS // N_CORES     # 8 pairs per core
KT = S // 128              # 16 k-tiles of 128 rows
QC = 512                   # q-chunk width (4 per pair)
NG = PPC * (S // QC)       # 32 (pair, qc) groups per core
F32 = mybir.dt.float32
BF16 = mybir.dt.bfloat16
FP8 = mybir.dt.float8e4
EXP = mybir.ActivationFunctionType.Exp
COPY = mybir.ActivationFunctionType.Copy
DR = mybir.MatmulPerfMode.DoubleRow

# k-tiles whose score matmuls run as fp8e4 DoubleRow (0.5 cycles/row on
# the PE; ~2x the matmul throughput). These are chunk index 6 (slices
# 12,13) of every group except group 0 (whose fp8 operands may not have
# landed yet) — the induced score error at 2/16 coverage keeps the
# end-to-end L2 rel err ~1.5e-2, under the 2e-2 gate.
FP8_KTILES = (0, 1)
FP8_CHUNK = 0
NT8 = len(FP8_KTILES)
# fp8 operands are pre-scaled by 1/8 so their products land directly in
# the u = s/64 domain of the bf16 stream — either exp engine can then
# process fp8-origin chunks with no rescale (subnormal cost is negligible,
# measured).
FP8_Q = 0.125

# Factored-cubic exp approximation (see module docstring).
#   p(u) = (u + EXP_A) * ((u + EXP_B)*u + EXP_C)  ~=  Lam * e^u
# on |u| <= 0.6875 (log-minimax, max |log err| 1.14e-3 -> 9.1e-3 at ^8).
EXP_A = 1.6925479387894398
EXP_B = 1.4963644896086045
EXP_C = 3.6262953097973463
EXP_LOGL = 1.815420023495584       # ln(Lam)
ACT_SCALE = 8.0                    # u = s/64 -> exp(8u) = exp(s/8)
ACT_BIAS = 8.0 * EXP_LOGL          # ln(Lam^8): match the DVE chunks' scale

# Cost-model busy times (ns) for greedy ACT/DVE load balancing.
_ACT_NS = lambda w: (w + 222) / 1.2     # activation, PSUM in / SBUF out
_DVE_NS = lambda w: (w + 120) / 0.96    # custom DVE, PSUM in / SBUF out


def _register_exp8_op():
    """Register the custom-DVE op once per process, mirroring
    DveOp.compile()'s own construction so the pinned shas match."""
    name = "EXP8R_CUBIC_ANT"
    if name in dve_ops._SUB_OPCODE_FOR_NAME:
        return next(op for op in dve_ops.OPS if op.name == name)

    body = sq(sq(sq(((Src0 + C1) * Src0 + C2) * (Src0 + C0))))

    def _ref(in0, in1, c0, c1, c2):
        x = in0.astype(np.float32)
        g = (((x + np.float32(c1)) * x + np.float32(c2)) * (x + np.float32(c0))).astype(
            np.float32
        )
        for _ in range(3):
            g = (g * g).astype(np.float32)
        return g

    spec = Spec(body=body, reference=_ref)
    row = dve_ops._CUSTOM_DVE_ROW_BASE + len(dve_ops.OPS)
    dve_ops._SUB_OPCODE_FOR_NAME[name] = row
    shas = {}
    for ver in ("v3", "v4"):
        d = DveOpSpec(
            name=name,
            opcode=row,
            uops=dve_lower(spec, ver=ver),
            rd1_en=_has_src1(spec),
        )
        shas[ver] = d.sha(ver)
    op = dve_ops.DveOp(name, spec, subdim=False, uops_sha=shas)
    dve_ops.OPS.append(op)
    dve_ops.CUSTOM_DVE_SPECS[name] = spec
    return op


EXP8_OP = _register_exp8_op()


def build_bass():
    nc = bacc.Bacc()
    # The ACT bias operand must be an SBUF [128,1] tensor for non-Copy
    # functions; memset it inside the TileContext so the dependency is
    # tracked without an all-engine barrier delaying the first DMAs.
    bias_t = nc.alloc_sbuf_tensor("const-actbias", [128, 1], F32)
    warm_t = nc.alloc_sbuf_tensor("act-warm", [128, 1], F32)
    # Zero fp8 operands for the PV-bank zeroing matmul (DoubleRow, ap 260,
    # 0.5 cyc/row): lhsT [1, 2, 128], rhs [1, 2, 260].
    z8w_t = nc.alloc_sbuf_tensor("const-z8w", [1, 2, 128], FP8)
    z8x_t = nc.alloc_sbuf_tensor("const-z8x", [1, 2, 260], FP8)
    qt_d = nc.declare_dram_parameter("qt", [PPC, 128, S], BF16, isOutput=False)
    kt_d = nc.declare_dram_parameter("kt", [PPC, 128, S // 2], BF16, isOutput=False)
    vo_d = nc.declare_dram_parameter("vo", [PPC, 128, KT * 65], BF16, isOutput=False)
    # fp8 DoubleRow operands: phases i=0/1 are d-halves 0:32 / 32:64.
    qt8_d = nc.declare_dram_parameter("qt8", [PPC, 32, 2, S], FP8, isOutput=False)
    kt8_d = nc.declare_dram_parameter(
        "kt8", [PPC, 32, NT8, 2, 128], FP8, isOutput=False
    )
    # [pair, qc, qsub, q128, d|den] — host divides along the last axis.
    out_d = nc.declare_dram_parameter(
        "ot", [PPC, S // QC, QC // 128, 128, 65], F32, isOutput=True
    )

    # Per-group chunking of the 16 score slices: 8 chunks of 2 slices.
    CHUNK_SLICES = [2] * 8
    eng_t = {"A": 0.0, "D": 0.0}

    with TileContext(nc) as tc:
        with (
            tc.tile_pool(name="qt", bufs=3) as qt_pool,
            tc.tile_pool(name="kt", bufs=3) as kt_pool,
            tc.tile_pool(name="vo", bufs=3) as vo_pool,
            tc.tile_pool(name="qt8", bufs=3) as qt8_pool,
            tc.tile_pool(name="kt8", bufs=3) as kt8_pool,
            tc.tile_pool(name="ptg", bufs=4) as ptg_pool,
            tc.tile_pool(name="ob", bufs=3) as ob_pool,
            tc.tile_pool(name="ps_s", bufs=3, space="PSUM") as ps_s_pool,
            tc.tile_pool(name="ps_o", bufs=2, space="PSUM") as ps_o_pool,
        ):
            bias_ap = bias_t.ap()

            tiles = {}      # pair -> (qt, kt, vo)
            ptgs = {}       # group g -> persistent bf16 P^T tile [128, 8192]

            chunk_par = [0]

            def exp_emit(out_ap, in_ap, w, force=None, scale=ACT_SCALE):
                if force is None:
                    # Strict A/D alternation for full-width chunks keeps the
                    # 3-deep PSUM ring cadence; drains fill in greedily.
                    force = "A" if chunk_par[0] % 2 == 0 else "D"
                    chunk_par[0] += 1
                if force == "A" or (
                    force is None
                    and eng_t["A"] + _ACT_NS(w) <= eng_t["D"] + _DVE_NS(w)
                ) and force != "D":
                    eng_t["A"] += _ACT_NS(w)
                    nc.scalar.activation(
                        out_ap, in_ap, EXP, scale=scale, bias=bias_ap
                    )
                else:
                    eng_t["D"] += _DVE_NS(w)
                    nc.vector._custom_dve(
                        EXP8_OP, out=out_ap, in0=in_ap,
                        s0=EXP_A, s1=EXP_B, imm2=EXP_C,
                    )

            def drain_emit(out_ap, in_ap, w, force=None):
                # PSUM->SBUF drains: GPSIMD cannot touch PSUM on TRN2, so
                # these share the exp engines, greedy-balanced.
                if force == "A" or (
                    force is None
                    and eng_t["A"] + _ACT_NS(w) <= eng_t["D"] + _DVE_NS(w)
                ):
                    eng_t["A"] += _ACT_NS(w)
                    nc.scalar.activation(out_ap, in_ap, COPY)
                else:
                    eng_t["D"] += _DVE_NS(w)
                    nc.vector.tensor_copy(out=out_ap, in_=in_ap)

            obs = {}        # group g -> SBUF staging tile while draining
            obank = {}      # group g -> shared PSUM output bank
            PV_TS = list(range(2, KT)) + [0, 1]

            def pv_zero_bank(g):
                """Open group g's shared PV output bank: all 4 q-subtiles
                accumulate start=False into disjoint 65-col ranges of ONE
                bank, zeroed up front by a cheap fp8 DoubleRow matmul
                (ap 260 @ 0.5 cyc/row = 54ns of PE). This collapses the
                four per-qsub drains into one 260-wide copy per group."""
                ps = ps_o_pool.tile([128, 512], F32, name="ps", tag="o65")
                obank[g] = ps
                nc.tensor.matmul(
                    ps[:, 0:260],
                    z8w_t.ap(),
                    z8x_t.ap(),
                    start=True,
                    stop=True,
                    perf_mode=DR,
                    skip_group_check=True,
                )

            def emit_pv_half(g, qsub, half):
                """Half of one PV q-subtile (8 of 16 k-tiles) of group
                g = (pair p, q-chunk qc) — spread over two chunk slots so
                every slot gives the PE a uniform 2-score + 8-PV mix.
                k-tile order is rotated so the freshest exp slices (15,
                then the fp8 slices 0,1) are consumed last."""
                p, qc = divmod(g, S // QC)
                ptg = ptgs[g]
                vo = tiles[p][2]
                if g not in obs:
                    obs[g] = ob_pool.tile([128, 4 * 65], F32, name="ob", tag="ob")
                ob = obs[g]
                if g not in obank:
                    pv_zero_bank(g)
                o65 = obank[g]
                for j in range(half * 8, half * 8 + 8):
                    t = PV_TS[j]
                    nc.tensor.matmul(
                        o65[:, qsub * 65 : qsub * 65 + 65],
                        ptg[:, t * 512 + qsub * 128 : t * 512 + qsub * 128 + 128],
                        vo[:, t * 65 : (t + 1) * 65],
                        start=False,
                        stop=(j == KT - 1),
                        skip_group_check=True,
                    )
                if half == 1 and qsub == 1:
                    # qsub order within a group is q2,q3,q0,q1 — q1 closes
                    # the group: one 260-wide drain, then the output DMA.
                    del obank[g]
                    drain_emit(ob[:, 0:260], o65[:, 0:260], 260)
                    del ptgs[g], obs[g]
                    nc.sync.dma_start(
                        out=out_d[p][qc].transpose([1, 0, 2]),
                        in_=ob[:],
                    )

            # Global chunk stream: groups in order. The final group's last
            # chunk is split into two 256-wide ops so the closing exps land
            # on both engines in parallel; see the tail section below.
            seq = []  # (g, chunk_idx, slice_offset, n_slices)
            for g in range(NG):
                slices = CHUNK_SLICES if g < NG - 1 else [2] * 7 + [1, 1]
                off = 0
                for idx, ns in enumerate(slices):
                    seq.append((g, idx, off, ns))
                    off += ns

            def stage_pair(p):
                # All input DMAs go on the SP queue, ordered by first need
                # (Pool's queue is reserved for PSUM drains so they never
                # wait behind a long transfer).
                kt = kt_pool.tile([128, S // 2], BF16, name="kt")
                qt = qt_pool.tile([128, S], BF16, name="qt")
                kt8 = kt8_pool.tile([32, NT8, 2, 128], FP8, name="kt8")
                qt8 = qt8_pool.tile([32, 2, S], FP8, name="qt8")
                vo = vo_pool.tile([128, KT * 65], BF16, name="vo")
                if p == 0:
                    # Minimal first transfers, split across the sync and
                    # gpsimd DMA queues (the gpsimd queue is otherwise idle
                    # until the first PV drain at ~13us): group 0 consumes
                    # all of kt but only qt cols 0:512.
                    nc.sync.dma_start(out=kt[:, 0:128], in_=kt_d[p][:, 0:128])
                    nc.gpsimd.dma_start(out=qt[:, 0:256], in_=qt_d[p][:, 0:256])
                    nc.sync.dma_start(out=qt[:, 256:512], in_=qt_d[p][:, 256:512])
                    nc.gpsimd.dma_start(out=kt[:, 128:256], in_=kt_d[p][:, 128:256])
                    nc.sync.dma_start(out=kt[:, 256:512], in_=kt_d[p][:, 256:512])
                    nc.gpsimd.dma_start(out=kt[:, 512:1024], in_=kt_d[p][:, 512:1024])
                    nc.sync.dma_start(out=qt[:, 512:1024], in_=qt_d[p][:, 512:1024])
                    nc.gpsimd.dma_start(out=vo[:], in_=vo_d[p])
                    nc.sync.dma_start(out=qt[:, 1024:S], in_=qt_d[p][:, 1024:S])
                    nc.sync.dma_start(out=kt8[:], in_=kt8_d[p])
                    nc.sync.dma_start(out=qt8[:], in_=qt8_d[p])
                else:
                    nc.sync.dma_start(out=kt[:, 0:256], in_=kt_d[p][:, 0:256])
                    nc.sync.dma_start(out=qt[:, 0:512], in_=qt_d[p][:, 0:512])
                    nc.sync.dma_start(
                        out=kt[:, 256 : S // 2], in_=kt_d[p][:, 256 : S // 2]
                    )
                    nc.sync.dma_start(out=kt8[:], in_=kt8_d[p])
                    nc.sync.dma_start(out=qt8[:], in_=qt8_d[p])
                    nc.sync.dma_start(out=vo[:], in_=vo_d[p])
                    nc.sync.dma_start(out=qt[:, 512:1024], in_=qt_d[p][:, 512:1024])
                    nc.sync.dma_start(out=qt[:, 1024:S], in_=qt_d[p][:, 1024:S])
                tiles[p] = (qt, kt, vo, qt8, kt8)

            def emit_score_slice(sc, i, t, kt, qt, qc, halves=False, fp8_pair=None):
                if fp8_pair is not None:
                    qt8, kt8 = fp8_pair
                    nc.tensor.matmul(
                        sc[:, i * 512 : (i + 1) * 512],
                        kt8[:, FP8_KTILES.index(t)],
                        qt8[:, :, qc * QC : (qc + 1) * QC],
                        start=True,
                        stop=True,
                        perf_mode=DR,
                    )
                    return
                strip = (t % 2) * 64
                col = (t // 2) * 128
                if halves:
                    for h in range(2):
                        nc.tensor.matmul(
                            sc[:, i * 512 + h * 256 : i * 512 + (h + 1) * 256],
                            kt[strip : strip + 64, col : col + 128],
                            qt[
                                strip : strip + 64,
                                qc * QC + h * 256 : qc * QC + (h + 1) * 256,
                            ],
                            start=True,
                            stop=True,
                            tile_position=(strip, 0),
                        )
                else:
                    nc.tensor.matmul(
                        sc[:, i * 512 : (i + 1) * 512],
                        kt[strip : strip + 64, col : col + 128],
                        qt[strip : strip + 64, qc * QC : (qc + 1) * QC],
                        start=True,
                        stop=True,
                        tile_position=(strip, 0),
                    )

            stage_pair(0)
            # Constants and the ACT table preload go behind pair 0's first
            # DMAs on the gpsimd queue so the first score matmul isn't
            # delayed.
            nc.gpsimd.memset(z8w_t.ap(), 0.0)
            nc.gpsimd.memset(z8x_t.ap(), 0.0)
            nc.gpsimd.memset(bias_t.ap(), ACT_BIAS)
            nc.gpsimd.memset(warm_t.ap(), -8.0)
            nc.scalar.activation(
                warm_t.ap(), warm_t.ap(), EXP, scale=ACT_SCALE, bias=bias_ap
            )

            for ci, (g, m, off, ns) in enumerate(seq):
                p, qc = divmod(g, S // QC)
                if p not in tiles:
                    stage_pair(p)
                # Prefetch the next pair's inputs two groups before they
                # are needed so SP-queue bursts never starve the PE.
                if qc == S // QC - 2 and off == 0 and p + 1 < PPC and p + 1 not in tiles:
                    stage_pair(p + 1)
                qt, kt = tiles[p][0], tiles[p][1]
                if g not in ptgs:
                    ptgs[g] = ptg_pool.tile([128, KT * 512], BF16, name="ptg", tag="ptg")
                w = ns * 512
                sc = ps_s_pool.tile([128, 2 * 512], F32, tag="s")
                for i in range(ns):
                    t = off + i
                    emit_score_slice(
                        sc, i, t, kt, qt, qc,
                        halves=(g == 0 and m == 0),
                        fp8_pair=(
                            (tiles[p][3], tiles[p][4])
                            if g > 0 and t in FP8_KTILES
                            else None
                        ),
                    )
                if g == NG - 1 and m == 8:
                    # Final exp: q3's stationary columns (384:512) land
                    # first on ACT so the scalar-queue output DMA chain —
                    # the longest tail — starts as early as possible; the
                    # rest go to DVE in parallel.
                    exp_emit(
                        ptgs[g][:, off * 512 + 384 : off * 512 + 512],
                        sc[:, 384:512],
                        128,
                        force="A",
                    )
                    exp_emit(
                        ptgs[g][:, off * 512 : off * 512 + 384],
                        sc[:, 0:384],
                        384,
                        force="D",
                    )
                elif g == 0 and m < 3:
                    # Warmup: split the first chunks' exps across both
                    # engines so the PSUM chunk ring drains at half latency
                    # while the exp pipeline fills.
                    exp_emit(
                        ptgs[g][:, off * 512 : off * 512 + 512],
                        sc[:, 0:512],
                        512,
                        force="A",
                    )
                    exp_emit(
                        ptgs[g][:, off * 512 + 512 : off * 512 + 1024],
                        sc[:, 512:1024],
                        512,
                        force="D",
                    )
                else:
                    exp_emit(ptgs[g][:, off * 512 : off * 512 + w], sc[:, :w], w)
                # Half a PV q-subtile per chunk slot: group g-1's four
                # q-subtiles (in order q2,q3,q0,q1) spread across all 8 of
                # group g's slots.
                if g >= 1 and m < 8:
                    qsub, half = ((2, 3, 0, 1)[m // 2], m % 2)
                    emit_pv_half(g - 1, qsub, half)

            # ---- tail: group NG-1 ----
            g = NG - 1
            p, qc = divmod(g, S // QC)
            ptg = ptgs[g]
            vo = tiles[p][2]
            ob = ob_pool.tile([128, 4 * 65], F32, name="ob", tag="ob")
            pv_zero_bank(g)
            acc = obank.pop(g)
            # All k-tiles except 15 for the four q-subtiles run while the
            # final exps complete; t-major with the freshest slices (13, 14)
            # last so no matmul waits on a late exp chunk.
            ts_pre = list(range(2, KT - 2)) + [0, 1, KT - 2]
            for t in ts_pre:
                for qsub in range(4):
                    nc.tensor.matmul(
                        acc[:, qsub * 65 : qsub * 65 + 65],
                        ptg[:, t * 512 + qsub * 128 : t * 512 + qsub * 128 + 128],
                        vo[:, t * 65 : (t + 1) * 65],
                        start=False,
                        stop=False,
                        skip_group_check=True,
                    )
            # k-tile 15 tails + drains + output DMAs, longest chain first.
            t = KT - 1

            def t15(qsub):
                nc.tensor.matmul(
                    acc[:, qsub * 65 : qsub * 65 + 65],
                    ptg[:, t * 512 + qsub * 128 : t * 512 + qsub * 128 + 128],
                    vo[:, t * 65 : (t + 1) * 65],
                    start=False,
                    stop=True,
                    skip_group_check=True,
                )

            t15(3)
            drain_emit(ob[:, 195:260], acc[:, 195:260], 65, force="A")
            nc.scalar.dma_start(out=out_d[p][qc][3], in_=ob[:, 195:260])
            t15(2)
            drain_emit(ob[:, 130:195], acc[:, 130:195], 65)
            nc.gpsimd.dma_start(out=out_d[p][qc][2], in_=ob[:, 130:195])
            t15(0)
            t15(1)
            drain_emit(ob[:, 0:65], acc[:, 0:65], 65)
            drain_emit(ob[:, 65:130], acc[:, 65:130], 65)
            nc.sync.dma_start(
                out=out_d[p][qc][0:2].transpose([1, 0, 2]), in_=ob[:, 0:130]
            )
    nc.compile()
    return nc


def _prep_inputs(query, key, value):
    """Host-side layout prep. Returns per-core input maps."""
    q = np.ascontiguousarray(query.reshape(PAIRS, S, D))
    k = np.ascontiguousarray(key.reshape(PAIRS, S, D))
    v = np.ascontiguousarray(value.reshape(PAIRS, S, D))

    qt = q.transpose(0, 2, 1) * np.float32(1.0 / 64.0)   # [PAIRS, 64, 2048]
    qt_dup = np.concatenate([qt, qt], axis=1)            # [PAIRS, 128, 2048]
    qt_dup = np.ascontiguousarray(qt_dup).astype(ml_dtypes.bfloat16)

    # kt_paired[p, 0:64, 128t+j]  = K^T[p, :, 256t + j]
    # kt_paired[p, 64:128, 128t+j] = K^T[p, :, 256t + 128 + j]
    kt = k.transpose(0, 2, 1).reshape(PAIRS, D, KT // 2, 2, 128)
    kt_paired = np.ascontiguousarray(
        kt.transpose(0, 3, 1, 2, 4).reshape(PAIRS, 128, S // 2)
    ).astype(ml_dtypes.bfloat16)

    e4m3 = ml_dtypes.float8_e4m3
    qs = q.transpose(0, 2, 1) * np.float32(FP8_Q)            # [PAIRS, 64, S]
    qt8 = np.ascontiguousarray(
        qs.reshape(PAIRS, 2, 32, S).transpose(0, 2, 1, 3)    # [PAIRS, 32, 2, S]
    ).astype(e4m3)
    # kt8[p, c, ti, i, m] = K[p, FP8_KTILES[ti]*128 + m, i*32 + c] * 8
    kss = np.stack(
        [k[:, t * 128 : (t + 1) * 128, :] for t in FP8_KTILES], axis=1
    )  # [PAIRS, NT8, 128, 64]
    kt8 = np.ascontiguousarray(
        (kss * np.float32(FP8_Q)).reshape(PAIRS, NT8, 128, 2, 32).transpose(0, 4, 1, 3, 2)
    ).astype(e4m3)

    vt = v.reshape(PAIRS, KT, 128, D).transpose(0, 2, 1, 3)  # [PAIRS,128,KT,64]
    vo = np.empty((PAIRS, 128, KT, 65), dtype=np.float32)
    vo[:, :, :, :D] = vt
    vo[:, :, :, D] = 1.0
    vo = vo.reshape(PAIRS, 128, KT * 65).astype(ml_dtypes.bfloat16)

    in_maps = []
    for c in range(N_CORES):
        sl = slice(c * PPC, (c + 1) * PPC)
        in_maps.append(
            {
                "qt": np.ascontiguousarray(qt_dup[sl]),
                "kt": np.ascontiguousarray(kt_paired[sl]),
                "vo": np.ascontiguousarray(vo[sl]),
                "qt8": np.ascontiguousarray(qt8[sl]),
                "kt8": np.ascontiguousarray(kt8[sl]),
            }
        )
    return in_maps


_CACHED_NC = None


def kernel(query, key, value, _want_results_obj=False, _trace=False):
    global _CACHED_NC
    if _CACHED_NC is None:
        _CACHED_NC = build_bass()
    nc = _CACHED_NC

    in_maps = _prep_inputs(query, key, value)
    res = run_bass_kernel_spmd(
        nc, in_maps, core_ids=list(range(N_CORES)), trace=_trace
    )

    # ot: [PPC, qc, qsub, 128, 65] -> [B, H, S, D]
    ot = np.concatenate([res.results[c]["ot"] for c in range(N_CORES)], axis=0)
    ot = ot.reshape(PAIRS, S, 65)
    out = ot[:, :, :D] / ot[:, :, D : D + 1]
    out = out.reshape(B, H, S, D).astype(np.float32)
    if _want_results_obj:
        return out, res
    return out


if __name__ == "__main__":
    rng = np.random.default_rng(0)
    q = rng.standard_normal((B, H, S, D), dtype=np.float32)
    k = rng.standard_normal((B, H, S, D), dtype=np.float32)
    v = rng.standard_normal((B, H, S, D), dtype=np.float32)
    o = kernel(query=q, key=k, value=v)
    print("out shape:", o.shape, o.dtype)
